# revision 8
# baseline (speedup 1.0000x reference)
"""DiagGCN message-passing kernel for 8 Trainium2 NeuronCores.

Strategy (receiver-sharded, no collectives):
  - Core c owns output rows [c*12500, (c+1)*12500). Edges are bucketed to
    cores by recv_idx, so each core computes its output slice completely.
  - Within a core, edges are ordered by (sender-chunk, 128-node window of
    the receiver). Sender chunks (4 x 25000 rows) keep dma_gather indices
    within int16 range. Subgroup sizes are padded to a cross-core-uniform
    schedule so one SPMD program serves all 8 cores; pad slots carry
    weight 0 and contribute nothing.
  - Per span of up to SPAN edge slots: dma_gather sender rows (512B each)
    and type rows from HBM into SBUF (edge e -> partition e%128).
  - DVE: msg = sender * type (+bias); ACT: relu; DVE: build weighted
    one-hot lhsT[e, m] = w[e] * (recv_inwin[e] == m) in ONE fused
    tensor_scalar(is_equal, mult) op per 128-edge tile.
  - PE: psum[window] (+)= onehot^T @ msg  (segment-sum as matmul).
  - DVE drains each finished window from PSUM into an SBUF accumulator;
    one strided DMA writes the [12500, 128] slice at the end.
"""
import sys
sys.path.insert(0, "/opt/trn_rl_repo")
import numpy as np
from dataclasses import dataclass


@dataclass(frozen=True)
class Config:
    n_nodes: int = 100000
    n_edges: int = 600000
    d: int = 128
    n_types: int = 401
    n_cores: int = 8
    chunks: int = 4          # sender-table chunks (int16 idx limit)
    span: int = 1024         # edge slots per gather call / compute span

    @property
    def npc(self):           # nodes per core
        return self.n_nodes // self.n_cores

    @property
    def nwin(self):          # 128-node windows per core
        return (self.npc + 127) // 128

    @property
    def crows(self):         # sender rows per chunk
        return (self.n_nodes + self.chunks - 1) // self.chunks


CFG = Config()

_PROGRAM_CACHE = {}


def _wrap16(arr):
    """[NC, L] int -> [NC, 128, L/16] int16: idx j at [:, j%16, j//16], x8."""
    nc_, L = arr.shape
    a = arr.astype(np.int16).reshape(nc_, L // 16, 16).transpose(0, 2, 1)
    return np.ascontiguousarray(np.tile(a, (1, 8, 1)))


def _wrap128(arr):
    """[NC, L] f32 -> [NC, 128, L/128]: slot j at [:, j%128, j//128]."""
    nc_, L = arr.shape
    a = arr.astype(np.float32).reshape(nc_, L // 128, 128).transpose(0, 2, 1)
    return np.ascontiguousarray(a)


def _schedule(S, cfg):
    """Static schedule from padded subgroup sizes S [chunks, nwin]."""
    chunks, nwin = S.shape
    offs = np.concatenate([[0], np.cumsum(S.ravel())])[:-1].reshape(chunks, nwin)
    L = int(S.sum())
    # window modes: first nonempty chunk copies, later chunks add
    first_chunk = np.full(nwin, -1, np.int64)
    for c in range(chunks):
        m = (S[c] > 0) & (first_chunk < 0)
        first_chunk[m] = c
    memset_windows = [w for w in range(nwin) if first_chunk[w] < 0]

    spans = []   # (slot_off, n_slots, chunk, span_tiles)
    gw = -1
    windows = []  # per nonempty (c,w): dict(c,w,gw,mode)
    for c in range(chunks):
        Lc = int(S[c].sum())
        if Lc == 0:
            continue
        c_off = int(offs[c, 0])
        # tiles of this chunk in order, annotated with window + first/last
        tiles = []
        for w in range(nwin):
            nt = int(S[c, w]) // 128
            if nt == 0:
                continue
            gw += 1
            windows.append(dict(c=c, w=w, gw=gw,
                                mode="copy" if first_chunk[w] == c else "add"))
            for k in range(nt):
                tiles.append(dict(w=w, gw=gw, first=(k == 0), last=(k == nt - 1)))
        # split into spans
        pos = 0
        while pos < Lc:
            n = min(cfg.span, Lc - pos)
            t0 = pos // 128
            spans.append(dict(off=c_off + pos, n=n, chunk=c,
                              tiles=tiles[t0:t0 + n // 128]))
            pos += n
    # windows ending per span index
    for s, sp in enumerate(spans):
        sp["ending"] = [t["gw"] for t in sp["tiles"] if t["last"]]
    return dict(spans=spans, windows=windows, memset_windows=memset_windows,
                L=L, offs=offs, n_windows=gw + 1)


def _build_program(S_bytes, L, has_bias, cfg):
    import concourse.bacc as bacc
    import concourse.bass as bass
    import concourse.mybir as mybir
    from concourse.library_config import mlp

    S = np.frombuffer(S_bytes, np.int64).reshape(cfg.chunks, cfg.nwin)
    sch = _schedule(S, cfg)
    spans, windows = sch["spans"], sch["windows"]
    nspan = len(spans)
    n_windows = sch["n_windows"]
    NWIN, NPC, D = cfg.nwin, cfg.npc, cfg.d
    SPAN_T = cfg.span // 128
    f32 = mybir.dt.float32

    nc = bacc.Bacc("TRN2", debug=True)
    vtab = nc.dram_tensor("vtab", [cfg.n_nodes, D], f32, kind="ExternalInput")
    vtyp = nc.dram_tensor("vtyp", [cfg.n_types, D], f32, kind="ExternalInput")
    sidx_d = nc.dram_tensor("sidx", [128, L // 16], mybir.dt.int16, kind="ExternalInput")
    tidx_d = nc.dram_tensor("tidx", [128, L // 16], mybir.dt.int16, kind="ExternalInput")
    recvf_d = nc.dram_tensor("recvf", [128, L // 128], f32, kind="ExternalInput")
    wf_d = nc.dram_tensor("wf", [128, L // 128], f32, kind="ExternalInput")
    iota_d = nc.dram_tensor("iota", [128, 128], f32, kind="ExternalInput")
    if has_bias:
        brep_d = nc.dram_tensor("brep", [128, D], f32, kind="ExternalInput")
    out_d = nc.dram_tensor("out", [NPC, D], f32, kind="ExternalOutput")

    from contextlib import ExitStack
    with ExitStack() as ctx:
        sidx_t = ctx.enter_context(nc.sbuf_tensor("sidx_t", [128, L // 16], mybir.dt.int16))
        tidx_t = ctx.enter_context(nc.sbuf_tensor("tidx_t", [128, L // 16], mybir.dt.int16))
        recvf_t = ctx.enter_context(nc.sbuf_tensor("recvf_t", [128, L // 128], f32))
        wf_t = ctx.enter_context(nc.sbuf_tensor("wf_t", [128, L // 128], f32))
        iota_t = ctx.enter_context(nc.sbuf_tensor("iota_t", [128, 128], f32))
        brep_t = ctx.enter_context(nc.sbuf_tensor("brep_t", [128, D], f32))
        sbuf0 = ctx.enter_context(nc.sbuf_tensor("sbuf0", [128, SPAN_T, D], f32))
        sbuf1 = ctx.enter_context(nc.sbuf_tensor("sbuf1", [128, SPAN_T, D], f32))
        tbuf0 = ctx.enter_context(nc.sbuf_tensor("tbuf0", [128, SPAN_T, D], f32))
        tbuf1 = ctx.enter_context(nc.sbuf_tensor("tbuf1", [128, SPAN_T, D], f32))
        ohbuf0 = ctx.enter_context(nc.sbuf_tensor("ohbuf0", [128, SPAN_T, 128], f32))
        ohbuf1 = ctx.enter_context(nc.sbuf_tensor("ohbuf1", [128, SPAN_T, 128], f32))
        accum = ctx.enter_context(nc.sbuf_tensor("accum", [128, NWIN * 128], f32))
        psum = ctx.enter_context(nc.psum_tensor("psum", [128, 8, 512], f32))
        ld = ctx.enter_context(nc.semaphore("ld"))
        gs = ctx.enter_context(nc.semaphore("gs"))
        vm = ctx.enter_context(nc.semaphore("vm"))
        ar = ctx.enter_context(nc.semaphore("ar"))
        ohs = ctx.enter_context(nc.semaphore("ohs"))
        mm = ctx.enter_context(nc.semaphore("mm"))
        rs = ctx.enter_context(nc.semaphore("rs"))
        pes = ctx.enter_context(nc.semaphore("pes"))
        od = ctx.enter_context(nc.semaphore("od"))
        block = ctx.enter_context(nc.Block())
        sbufs, tbufs, ohbufs = [sbuf0, sbuf1], [tbuf0, tbuf1], [ohbuf0, ohbuf1]
        n_loads = 6 if has_bias else 5

        @block.gpsimd
        def _(g):
            g.load_library(mlp)
            g.dma_start(sidx_t[:], sidx_d[:]).then_inc(ld, 16)
            g.dma_start(tidx_t[:], tidx_d[:]).then_inc(ld, 16)
            g.dma_start(recvf_t[:], recvf_d[:]).then_inc(ld, 16)
            g.dma_start(wf_t[:], wf_d[:]).then_inc(ld, 16)
            g.dma_start(iota_t[:], iota_d[:]).then_inc(ld, 16)
            if has_bias:
                g.dma_start(brep_t[:], brep_d[:]).then_inc(ld, 16)
            g.wait_ge(ld, 16 * n_loads)
            call_i = 0
            for s, sp in enumerate(spans):
                if s >= 2:
                    g.wait_ge(pes, s - 1)
                k, n, off, c = s % 2, sp["n"], sp["off"], sp["chunk"]
                nt = n // 128
                cr0 = c * cfg.crows
                crn = min(cfg.crows, cfg.n_nodes - cr0)
                # SWDGE ring holds ~2048 descriptors: keep <=2 gather calls
                # (1024 descs each) issued-but-unfinished at any time
                if call_i >= 1:
                    g.wait_ge(gs, 16 * call_i)
                g.dma_gather(
                    sbufs[k][:, :nt, :], vtab[cr0:cr0 + crn, :],
                    sidx_t[:, off // 16:(off + n) // 16], n, n, D,
                ).then_inc(gs, 16)
                call_i += 1
                if call_i >= 1:
                    g.wait_ge(gs, 16 * call_i)
                g.dma_gather(
                    tbufs[k][:, :nt, :], vtyp[:, :],
                    tidx_t[:, off // 16:(off + n) // 16], n, n, D,
                ).then_inc(gs, 16)
                call_i += 1
            # final output store
            g.wait_ge(rs, n_windows)
            full_w = NWIN - 1 if NPC % 128 else NWIN
            g.dma_start(
                out_d[0:full_w * 128, :].rearrange("(w p) d -> p w d", p=128),
                accum[:, 0:full_w * 128].rearrange("p (w d) -> p w d", d=D),
            ).then_inc(od, 16)
            if NPC % 128:
                rem = NPC - full_w * 128
                g.dma_start(
                    out_d[full_w * 128:NPC, :],
                    accum[0:rem, full_w * 128:full_w * 128 + D],
                ).then_inc(od, 16)
                g.wait_ge(od, 32)
            else:
                g.wait_ge(od, 16)

        @block.vector
        def _(v):
            v.wait_ge(ld, 16 * n_loads)
            for w in sch["memset_windows"]:
                v.memset(accum[:, w * 128:(w + 1) * 128], 0.0)
            for s, sp in enumerate(spans):
                k, n, off = s % 2, sp["n"], sp["off"]
                nt = n // 128
                if s >= 2:
                    v.wait_ge(pes, s - 1)
                v.wait_ge(gs, 32 * (s + 1))
                v.tensor_mul(sbufs[k][:, :nt, :], sbufs[k][:, :nt, :],
                             tbufs[k][:, :nt, :])
                if has_bias:
                    for i in range(nt):
                        v.tensor_add(sbufs[k][:, i, :], sbufs[k][:, i, :],
                                     brep_t[:, :].rearrange("p (o d) -> p o d", o=1))
                v.drain().then_inc(vm, 1)
                for i in range(nt):
                    col = off // 128 + i
                    v.tensor_scalar(
                        ohbufs[k][:, i, :],
                        iota_t[:, :].rearrange("p (o d) -> p o d", o=1),
                        recvf_t[:, col:col + 1],
                        wf_t[:, col:col + 1],
                        mybir.AluOpType.is_equal,
                        mybir.AluOpType.mult,
                    )
                v.drain().then_inc(ohs, 1)
                # drain windows whose accumulation finished in this span
                for wi in sp["ending"]:
                    win = windows[wi]
                    v.wait_ge(mm, wi + 1)
                    dst = accum[:, win["w"] * 128:win["w"] * 128 + D]
                    src = psum[:, wi % 8, 0:D]
                    if win["mode"] == "copy":
                        v.tensor_copy(dst, src)
                    else:
                        v.tensor_add(dst, dst, src)
                    v.drain().then_inc(rs, 1)

        @block.scalar
        def _(a):
            for s, sp in enumerate(spans):
                k, nt = s % 2, sp["n"] // 128
                a.wait_ge(vm, s + 1)
                a.activation(sbufs[k][:, :nt, :], sbufs[k][:, :nt, :],
                             mybir.ActivationFunctionType.Relu)
                a.drain().then_inc(ar, 1)

        @block.tensor
        def _(t):
            for s, sp in enumerate(spans):
                k = s % 2
                t.wait_ge(ar, s + 1)
                t.wait_ge(ohs, s + 1)
                for i, tile in enumerate(sp["tiles"]):
                    gw = tile["gw"]
                    if tile["first"] and gw >= 8:
                        t.wait_ge(rs, gw - 7)
                    inst = t.matmul(
                        psum[:, gw % 8, 0:D],
                        ohbufs[k][:, i, :],
                        sbufs[k][:, i, :],
                        start=tile["first"], stop=tile["last"],
                    )
                    if tile["last"]:
                        inst.then_inc(mm, 1)
                t.drain().then_inc(pes, 1)

    nc.compile()
    return nc


def _get_program(S, L, has_bias, cfg):
    key = (S.tobytes(), L, has_bias, cfg)
    if key not in _PROGRAM_CACHE:
        _PROGRAM_CACHE[key] = _build_program(S.tobytes(), L, has_bias, cfg)
    return _PROGRAM_CACHE[key]


def _prepare(V, VT, B, w, snd, typ, rcv, cfg):
    NC, NPC, NWIN, CH = cfg.n_cores, cfg.npc, cfg.nwin, cfg.chunks
    E = cfg.n_edges
    snd = np.asarray(snd, np.int64)
    typ = np.asarray(typ, np.int64)
    rcv = np.asarray(rcv, np.int64)
    w = np.asarray(w, np.float32)

    core = rcv // NPC
    rloc = rcv - core * NPC
    win = rloc // 128
    rin = (rloc - win * 128).astype(np.float32)
    chunk = snd // cfg.crows
    sloc = (snd - chunk * cfg.crows).astype(np.int16)

    key = (core * CH + chunk) * NWIN + win
    order = np.argsort(key, kind="stable")
    cnt = np.bincount(key, minlength=NC * CH * NWIN).reshape(NC, CH, NWIN)
    S = ((cnt.max(axis=0) + 127) // 128 * 128).astype(np.int64)  # [CH, NWIN]
    offs = np.concatenate([[0], np.cumsum(S.ravel())])[:-1].reshape(CH, NWIN)
    L = int(S.sum())

    # slot for each edge (in sorted order)
    cnt_flat = cnt.ravel()
    grp_start = np.concatenate([[0], np.cumsum(cnt_flat)])[:-1]
    ranks = np.arange(E) - np.repeat(grp_start, cnt_flat)
    # per-edge (sorted) subgroup offset: offs[chunk, win] (same for all cores)
    ids = key[order]
    c_of = (ids // NWIN) % CH
    w_of = ids % NWIN
    slot = offs[c_of, w_of] + ranks
    core_s = core[order]

    sl_s = np.zeros((NC, L), np.int16)
    sl_t = np.zeros((NC, L), np.int16)
    sl_r = np.zeros((NC, L), np.float32)
    sl_w = np.zeros((NC, L), np.float32)
    sl_s[core_s, slot] = sloc[order]
    sl_t[core_s, slot] = typ[order].astype(np.int16)
    sl_r[core_s, slot] = rin[order]
    sl_w[core_s, slot] = w[order]

    iota = np.ascontiguousarray(
        np.tile(np.arange(128, dtype=np.float32), (128, 1)))
    has_bias = bool(np.any(np.asarray(B) != 0))
    in_maps = []
    sidx_w = _wrap16(sl_s)
    tidx_w = _wrap16(sl_t)
    recvf = _wrap128(sl_r)
    wf = _wrap128(sl_w)
    Vc = np.ascontiguousarray(np.asarray(V, np.float32))
    VTc = np.ascontiguousarray(np.asarray(VT, np.float32))
    for i in range(NC):
        m = {"vtab": Vc, "vtyp": VTc, "sidx": sidx_w[i], "tidx": tidx_w[i],
             "recvf": recvf[i], "wf": wf[i], "iota": iota}
        if has_bias:
            m["brep"] = np.ascontiguousarray(
                np.tile(np.asarray(B, np.float32), (128, 1)))
        in_maps.append(m)
    return S, L, has_bias, in_maps


def _run(V, VT, B, w, snd, typ, rcv, cfg=None, trace=False):
    from concourse.bass_utils import run_bass_kernel_spmd
    cfg = cfg or CFG
    S, L, has_bias, in_maps = _prepare(V, VT, B, w, snd, typ, rcv, cfg)
    nc = _get_program(S, L, has_bias, cfg)
    res = run_bass_kernel_spmd(nc, in_maps, list(range(cfg.n_cores)),
                               trace=trace)
    out = np.concatenate([res.results[i]["out"] for i in range(cfg.n_cores)], 0)
    return out[:cfg.n_nodes], res


def kernel(V_proj_sender, V_types, B_message, inc_weights,
           sender_idx, type_idx, recv_idx):
    out, _ = _run(V_proj_sender, V_types, B_message, inc_weights,
                  sender_idx, type_idx, recv_idx)
    return out
